# revision 4
# baseline (speedup 1.0000x reference)
"""EquiNN forward on 8 TRN2 NeuronCores.

out[b, i, j] = l * X[b, i, j] + g * sum_k X[b, i, k]

Sharding: pure data parallel — X (8, 2048, 2048) f32 is split along the
leading batch dim, one (2048, 2048) slab per core; scalars l, g are
replicated to every core.

Per-core kernel (memory-bound, ~32 MiB HBM traffic per core):
  - view the slab as chunks of (128 partitions, R rows, 2048) so each
    partition holds R whole rows
  - DMA chunk in (sync/SP HWDGE ring), reduce_sum along the row axis,
    fuse out = (x * l) + (g * rowsum) into one DVE tensor_scalar pass
    (per-partition scalar operands), DMA chunk out (scalar/ACT HWDGE
    ring so loads and stores live on independent rings)
  - l, g are broadcast to all 128 partitions once via gpsimd
    partition_broadcast
"""

import numpy as np

import concourse.bacc as bacc
import concourse.mybir as mybir
import concourse.tile as tile
from concourse.bass_utils import run_bass_kernel_spmd

B = 8          # batch == number of cores
N = 2048       # rows per slab
M = 2048       # row length
P = 128        # SBUF partitions
R = 2          # rows per partition per chunk -> chunks of P*R rows

F32 = mybir.dt.float32

# test-harness hooks (the grading harness just calls kernel())
TRACE = False
LAST_RESULT = None

_cached_nc = None


def _build():
    nc = bacc.Bacc("TRN2", target_bir_lowering=False, debug=False)
    x = nc.dram_tensor("x", [N, M], F32, kind="ExternalInput")
    l = nc.dram_tensor("l", [1, 1], F32, kind="ExternalInput")
    g = nc.dram_tensor("g", [1, 1], F32, kind="ExternalInput")
    y = nc.dram_tensor("y", [N, M], F32, kind="ExternalOutput")

    n_chunks = N // (P * R)
    xv = x[:, :].rearrange("(c p r) m -> c p r m", p=P, r=R)
    yv = y[:, :].rearrange("(c p r) m -> c p r m", p=P, r=R)

    with tile.TileContext(nc) as tc:
        with (
            tc.tile_pool(name="const", bufs=1) as cpool,
            tc.tile_pool(name="io", bufs=3) as iopool,
            tc.tile_pool(name="stat", bufs=4) as spool,
        ):
            # lg[:, 0] = l, lg[:, 1] = g on every partition
            lg0 = cpool.tile([1, 2], F32)
            nc.sync.dma_start(out=lg0[:, 0:1], in_=l[:, :])
            nc.sync.dma_start(out=lg0[:, 1:2], in_=g[:, :])
            lg = cpool.tile([P, 2], F32)
            nc.gpsimd.partition_broadcast(lg[:], lg0[:])

            for c in range(n_chunks):
                t = iopool.tile([P, R, M], F32)
                nc.sync.dma_start(out=t[:], in_=xv[c])

                s = spool.tile([P, R], F32)
                nc.vector.reduce_sum(s[:], t[:], axis=mybir.AxisListType.X)
                gs = spool.tile([P, R], F32)
                nc.vector.tensor_scalar_mul(gs[:], s[:], lg[:, 1:2])

                o = iopool.tile([P, R, M], F32)
                for r in range(R):
                    nc.vector.tensor_scalar(
                        o[:, r, :],
                        t[:, r, :],
                        lg[:, 0:1],
                        gs[:, r : r + 1],
                        mybir.AluOpType.mult,
                        mybir.AluOpType.add,
                    )
                nc.scalar.dma_start(out=yv[c], in_=o[:])
    nc.compile()
    return nc


def kernel(X: np.ndarray, l: np.ndarray, g: np.ndarray) -> np.ndarray:
    global _cached_nc, LAST_RESULT
    assert X.shape == (B, N, M), X.shape
    if _cached_nc is None:
        _cached_nc = _build()
    nc = _cached_nc

    X = np.ascontiguousarray(X, dtype=np.float32)
    l2 = np.ascontiguousarray(l, dtype=np.float32).reshape(1, 1)
    g2 = np.ascontiguousarray(g, dtype=np.float32).reshape(1, 1)
    in_maps = [{"x": X[k], "l": l2, "g": g2} for k in range(B)]

    res = run_bass_kernel_spmd(nc, in_maps, core_ids=list(range(B)), trace=TRACE)
    LAST_RESULT = res
    return np.stack([res.results[k]["y"] for k in range(B)], axis=0)
